# revision 1
# baseline (speedup 1.0000x reference)
"""AdaptiveAttention Trainium2 kernel (8 NeuronCores, pure data parallel).

Reference computation per batch row b (N=49 spatial slots, D=512 hidden):
    image[b,n,k] = sum_d V[b,n,d] Wv[d,k]
    gh[b,n]      = sum_d h[b,d]  Wg[d,n]
    z[b,n]  = sum_k tanh(image[b,n,k] + gh[b,n]) wh[k]
    as[b]   = sum_k tanh(sum_d s[b,d] Ws[d,k] + gh[b,k]) wh[k]
    S50     = sum_n exp(z[b,n]) + exp(as[b])
    out[b,:] = (exp(as[b])/S50) s[b,:] + sum_n (exp(z[b,n])/S50) V[b,n,:]

Device strategy per core (1024 rows), all-fp16 data path (rel err ~2e-3):
  pass 1: stream V^T (d on partitions, fp16) -> image matmuls -> +gh
          broadcast (ones matmul) -> tanh -> z matmul.
  softmax pieces + a PE-transpose trick build per-pair stationaries
  [gamma*e ; beta] in SBUF.
  pass 2: stream natural V rows (fp16) with s rows interleaved; one
          block-diagonal matmul per row pair directly produces the final
          output rows.
All heavy layout work (transposes, splits, tiling) is done host-side in
numpy; device sees DMA-friendly contiguous slabs.
"""

import numpy as np

F16 = np.float16
F32 = np.float32

B = 8192
CORES = 8
BL = B // CORES          # 1024 rows per core
N = 49
D = 512
NCH = 4                  # d chunks of 128
NT = BL // 8             # 128 V^T tiles of 8 rows (392 bn-cols)
NP = NT // 2             # 64 tile pairs (t, t+64)
TW = 8 * N               # 392
EXP_SHIFT = 20.0         # exp(x - 20): keeps exp in fp32 range, ratios invariant


def build(stage=3, np_pairs=NP):
    nc = _build(stage, np_pairs)
    nc.compile()
    return nc


def _build(stage=3, np_pairs=NP):
    import concourse.bass as bass
    import concourse.mybir as mybir
    from concourse import bacc
    from concourse.tile import TileContext
    from concourse.masks import make_identity

    dt = mybir.dt
    ADD = mybir.AluOpType.add
    MULT = mybir.AluOpType.mult
    TANH = mybir.ActivationFunctionType.Tanh
    EXP = mybir.ActivationFunctionType.Exp

    nc = bacc.Bacc(num_devices=CORES)

    # ---- dram parameters (per-core shards; host does all layout) ----
    vt = nc.declare_dram_parameter("vt", [NP // 2, 128, 4 * NCH * TW], dt.float16, isOutput=False)
    vs = nc.declare_dram_parameter("vs", [32, 100, 8192], dt.float16, isOutput=False)
    sth = nc.declare_dram_parameter("sth", [128, NCH * 1024], dt.float16, isOutput=False)
    hth = nc.declare_dram_parameter("hth", [128, NCH * 1024], dt.float16, isOutput=False)
    wvh = nc.declare_dram_parameter("wvh", [128, NCH * 64], dt.float16, isOutput=False)
    wgh = nc.declare_dram_parameter("wgh", [128, NCH * 64], dt.float16, isOutput=False)
    wsh = nc.declare_dram_parameter("wsh", [128, NCH * 64], dt.float16, isOutput=False)
    wh2 = nc.declare_dram_parameter("wh2", [128, 2], dt.float16, isOutput=False)
    wh1 = nc.declare_dram_parameter("wh1", [64, 1], dt.float32, isOutput=False)
    outp = nc.declare_dram_parameter("out", [4, 2, 128, 512], dt.float16, isOutput=True)

    zscr = nc.dram_tensor("zscr", [16, 8, TW], dt.float32)
    ghd = nc.dram_tensor("ghd", [1024, 49], dt.float16)
    ascr = nc.dram_tensor("ascr", [1024], dt.float32)

    with TileContext(nc) as tc:
        with (
            tc.tile_pool(name="const", bufs=1) as cp,
            tc.tile_pool(name="misc", bufs=1) as mp,
            tc.tile_pool(name="vsp", bufs=3) as vsp,
            tc.tile_pool(name="work", bufs=3) as wp,
        ):
            # ---------- constants ----------
            t_wvh = cp.tile([128, NCH * 64], dt.float16, tag="wvh")
            t_wgh = cp.tile([128, NCH * 64], dt.float16, tag="wgh")
            t_wsh = cp.tile([128, NCH * 64], dt.float16, tag="wsh")
            t_wh2 = cp.tile([128, 2], dt.float16, tag="wh2")
            t_wh1 = cp.tile([64, 1], dt.float32, tag="wh1")
            ident = cp.tile([128, 128], dt.float32, tag="ident")
            for t, p in [(t_wvh, wvh), (t_wgh, wgh), (t_wsh, wsh),
                         (t_wh2, wh2), (t_wh1, wh1)]:
                nc.scalar.dma_start(out=t[:], in_=p[:])
            make_identity(nc, ident[:])

            shift_b = mp.tile([128, 1], dt.float32, tag="shiftb")
            nc.vector.memset(shift_b[:], -EXP_SHIFT)
            ones1 = mp.tile([98, 64], dt.float16, tag="ones1")
            nc.vector.memset(ones1[:], 1.0)
            gh_tsb = mp.tile([64, 1024], dt.float32, tag="ghtsb")
            ghz = mp.tile([98, 32 * TW], dt.float16, tag="ghz")
            as_sb = mp.tile([1, 1024], dt.float32, tag="assb")

            # ---------- prologue: gh (for z path) and attn_s ----------
            with (
                tc.tile_pool(name="shpool", bufs=1) as shp,
                tc.tile_pool(name="ppro", bufs=2, space="PSUM") as ppro,
            ):
                t_sth = shp.tile([128, NCH * 1024], dt.float16, tag="sth")
                t_hth = shp.tile([128, NCH * 1024], dt.float16, tag="hth")
                nc.sync.dma_start(out=t_sth[:], in_=sth[:])
                nc.sync.dma_start(out=t_hth[:], in_=hth[:])
                for grp in range(2):
                    # gh = h @ Wg  (fp16, accumulate in psum)
                    psg = ppro.tile([64, 512], dt.float32, tag="psg")
                    for k in range(4):
                        hh = t_hth[:, k * 1024 + 512 * grp: k * 1024 + 512 * grp + 512]
                        gh_k = t_wgh[:, 64 * k:64 * k + 64]
                        nc.tensor.matmul(out=psg[0:64, :], lhsT=gh_k, rhs=hh,
                                         start=(k == 0), stop=(k == 3))
                    # rows j=0..63 hold gh^T for this 512-col group (rows 49-63 zero)
                    nc.vector.tensor_copy(out=gh_tsb[0:64, 512 * grp:512 * grp + 512], in_=psg[0:64, :])

                    # attn pre-act: s @ Ws + h @ Wg
                    pst = ppro.tile([64, 512], dt.float32, tag="pst")
                    ops = []
                    for k in range(4):
                        co = k * 1024 + 512 * grp
                        ops += [
                            (t_wsh[:, 64 * k:64 * k + 64], t_sth[:, co:co + 512]),
                            (t_wgh[:, 64 * k:64 * k + 64], t_hth[:, co:co + 512]),
                        ]
                    for i, (lhsT, rhs) in enumerate(ops):
                        nc.tensor.matmul(out=pst[0:64, :], lhsT=lhsT, rhs=rhs,
                                         start=(i == 0), stop=(i == len(ops) - 1))
                    ts_sb = wp.tile([64, 512], dt.float32, tag="tssb")
                    nc.scalar.activation(out=ts_sb[:], in_=pst[0:64, :], func=TANH)
                    psa = ppro.tile([1, 512], dt.float32, tag="psa")
                    nc.tensor.matmul(out=psa[0:1, :], lhsT=t_wh1[:], rhs=ts_sb[:], start=True, stop=True)
                    nc.scalar.copy(out=as_sb[0:1, 512 * grp:512 * grp + 512], in_=psa[0:1, :])

            # gh^T [64j, 1024b] -> gh flat (b-major) in DRAM as fp16
            with tc.tile_pool(name="ptrg", bufs=2, space="PSUM") as ptrg:
                for i in range(8):
                    psT = ptrg.tile([128, 64], dt.float32, tag="psT")
                    nc.tensor.transpose(psT[0:128, :], gh_tsb[:, 128 * i:128 * i + 128], ident[0:64, 0:64])
                    ghh_sb = wp.tile([128, 64], dt.float16, tag="ghhsb")
                    nc.scalar.copy(out=ghh_sb[:], in_=psT[0:128, :])
                    nc.scalar.dma_start(out=ghd[128 * i:128 * i + 128], in_=ghh_sb[:, 0:N])
            # reload flat (b, n)-major: row 32r
            for r in range(4):
                nc.scalar.dma_start(
                    out=ghz[32 * r:32 * r + 1, :],
                    in_=ghd[256 * r:256 * r + 256, :].rearrange("a b -> (a b)").unsqueeze(0))

            # bounce as through dram to get [128, 8] layout
            nc.scalar.dma_start(out=ascr[:].rearrange("(o f) -> o f", o=1), in_=as_sb[0:1, :])
            as2 = mp.tile([128, 8], dt.float32, tag="as2")
            # permuted layout to match z_sb: partition p = 16g + 8u + j <-> tile t = 64u + 8g + j
            for g in range(8):
                for u in range(2):
                    nc.scalar.dma_start(
                        out=as2[16 * g + 8 * u:16 * g + 8 * u + 8, :],
                        in_=ascr[512 * u + 64 * g:512 * u + 64 * g + 64].rearrange("(j r) -> j r", r=8))

            # ---------- interleaved pass 1 / pass 2 ----------
            if stage < 1:
                nc.sync.dma_start(out=outp[0][0][:, 0:8], in_=as2[:])
                return nc
            # z_sb permuted layout: partition p = 16g + 8u + j  <->  tile t = 64u + 8g + j
            z_sb = mp.tile([128, TW], dt.float32, tag="zsb")
            e_sb = mp.tile([128, TW], dt.float32, tag="esb")
            escl = mp.tile([128, TW], dt.float32, tag="escl")
            s49 = mp.tile([128, 8], dt.float32, tag="s49")
            s50 = mp.tile([128, 8], dt.float32, tag="s50")
            recip = mp.tile([128, 8], dt.float32, tag="recip")
            beta_t = mp.tile([128, 8], dt.float32, tag="beta")
            e_as = mp.tile([128, 8], dt.float32, tag="eas")
            nc.scalar.activation(out=e_as[:], in_=as2[:], func=EXP, bias=shift_b[:])
            et0 = cp.tile([100, 256], dt.float16, tag="et0")
            et1 = cp.tile([100, 256], dt.float16, tag="et1")
            et2 = cp.tile([100, 256], dt.float16, tag="et2")
            et3 = cp.tile([100, 256], dt.float16, tag="et3")
            et = [et0, et1, et2, et3]

            with (
                tc.tile_pool(name="vtp", bufs=4) as vtp,
                tc.tile_pool(name="zstp", bufs=2) as zstp,
                tc.tile_pool(name="cstp", bufs=3) as cstp,
                tc.tile_pool(name="pimg", bufs=2, space="PSUM") as pimg,
                tc.tile_pool(name="pz", bufs=1, space="PSUM") as pz,
                tc.tile_pool(name="ptr", bufs=1, space="PSUM") as ptr,
                tc.tile_pool(name="pc", bufs=2, space="PSUM") as pc,
            ):
                z_stage = [None]
                vtile_ref = [None]
                HW_ = 2 * NCH * TW
                dmae = [nc.sync, nc.gpsimd]
                dmac = [0]
                cst = {}

                def emit_pair(sp, h2):
                    p = 2 * sp + h2
                    psi = pimg.tile([128, TW], dt.float32, tag="psi")
                    for u in range(2):
                        ro = 64 * u
                        t_idx = p + 64 * u
                        P, co = 32 * (t_idx // 32), (t_idx % 32) * TW
                        for k in range(4):
                            rhs = vtile_ref[0][:, h2 * HW_ + (u * 4 + k) * TW: h2 * HW_ + (u * 4 + k) * TW + TW]
                            nc.tensor.matmul(out=psi[ro:ro + 64, :], lhsT=t_wvh[:, 64 * k:64 * k + 64],
                                             rhs=rhs, start=(k == 0), stop=False)
                        nc.tensor.matmul(out=psi[ro:ro + 64, :], lhsT=ones1[P:P + 1, :],
                                         rhs=ghz[P:P + 1, co:co + TW], start=False, stop=True,
                                         tile_position=(P, ro))
                    th = wp.tile([128, TW], dt.float16, tag="th")
                    nc.scalar.activation(out=th[:], in_=psi[:, :], func=TANH)
                    psz = pz.tile([2, TW], dt.float32, tag="psz")
                    nc.tensor.matmul(out=psz[0:2, :], lhsT=t_wh2[:], rhs=th[:], start=True, stop=True)
                    off = (p % 8) * TW
                    nc.vector.tensor_copy(out=z_stage[0][0:2, off:off + TW], in_=psz[0:2, :])
                    if p % 8 == 7:
                        g = p // 8
                        nc.scalar.dma_start(out=zscr[2 * g:2 * g + 2],
                                            in_=z_stage[0][:, :].rearrange("u (t f) -> u t f", f=TW))
                        nc.scalar.dma_start(out=z_sb[16 * g:16 * g + 8, :], in_=zscr[2 * g])
                        nc.scalar.dma_start(out=z_sb[16 * g + 8:16 * g + 16, :], in_=zscr[2 * g + 1])

                def softmax_batch(G):
                    sl = slice(32 * G, 32 * G + 32)
                    nc.scalar.activation(out=e_sb[sl, :], in_=z_sb[sl, :], func=EXP, bias=shift_b[sl, :])
                    nc.vector.tensor_reduce(out=s49[sl, :], in_=e_sb[sl, :].rearrange("p (b n) -> p b n", n=N),
                                            axis=mybir.AxisListType.X, op=ADD)
                    nc.vector.tensor_tensor(out=s50[sl, :], in0=s49[sl, :], in1=e_as[sl, :], op=ADD)
                    nc.vector.reciprocal(out=recip[sl, :], in_=s50[sl, :])
                    nc.vector.tensor_tensor(out=beta_t[sl, :], in0=e_as[sl, :], in1=recip[sl, :], op=MULT)
                    nc.vector.tensor_tensor(
                        out=escl[sl, :].rearrange("p (b n) -> p b n", n=N),
                        in0=e_sb[sl, :].rearrange("p (b n) -> p b n", n=N),
                        in1=recip[sl, :].unsqueeze(2).broadcast_to([32, 8, N]),
                        op=MULT)

                def et_batch(G):
                    sl = slice(32 * G, 32 * G + 32)
                    for c in range(4):
                        esA = wp.tile([128, 100], dt.float32, tag="esA")
                        esB = wp.tile([128, 100], dt.float32, tag="esB")
                        nc.vector.memset(esA[sl, 49:100], 0.0)
                        nc.vector.memset(esB[sl, 0:50], 0.0)
                        nc.vector.tensor_copy(out=esA[sl, 0:49], in_=escl[sl, 98 * c:98 * c + 49])
                        nc.vector.tensor_copy(out=esA[sl, 49:50], in_=beta_t[sl, 2 * c:2 * c + 1])
                        nc.vector.tensor_copy(out=esB[sl, 50:99], in_=escl[sl, 98 * c + 49:98 * c + 98])
                        nc.vector.tensor_copy(out=esB[sl, 99:100], in_=beta_t[sl, 2 * c + 1:2 * c + 2])
                        psA = ptr.tile([100, 128], dt.float32, tag="psA")
                        psB = ptr.tile([100, 128], dt.float32, tag="psB")
                        nc.tensor.transpose(psA[0:100, :], esA[:, :], ident[:])
                        nc.tensor.transpose(psB[0:100, :], esB[:, :], ident[:])
                        etr = et[c][:, :].rearrange("p (t x) -> p t x", x=2)
                        for g in (2 * G, 2 * G + 1):
                            for u in range(2):
                                pcols = slice(16 * g + 8 * u, 16 * g + 8 * u + 8)
                                tcols = slice(64 * u + 8 * g, 64 * u + 8 * g + 8)
                                nc.vector.tensor_copy(out=etr[:, tcols, 0:1], in_=psA[0:100, pcols].unsqueeze(2))
                                nc.vector.tensor_copy(out=etr[:, tcols, 1:2], in_=psB[0:100, pcols].unsqueeze(2))

                def pass2_slab(qq):
                    vstile = vsp.tile([100, 8192], dt.float16, tag="vs")
                    dmae[dmac[0] % 2].dma_start(out=vstile[:], in_=vs[qq][:])
                    dmac[0] += 1
                    for j2 in range(2):
                        for g2 in range(2):
                            t = 4 * qq + 2 * j2 + g2
                            s8 = t // 8
                            if s8 not in cst:
                                cst[s8] = cstp.tile([98, 8 * 512], dt.float16, tag="cst", name="cstage")
                            cstage = cst[s8]
                            psc = pc.tile([128, 512], dt.float32, tag="psc")
                            for c in range(4):
                                nc.tensor.matmul(out=psc[32 * c:32 * c + 2, :],
                                                 lhsT=et[c][:, 2 * t:2 * t + 2],
                                                 rhs=vstile[:, j2 * 4096 + (g2 * 4 + c) * 512: j2 * 4096 + (g2 * 4 + c) * 512 + 512],
                                                 start=True, stop=True, tile_position=(0, 32 * c))
                            off = (t % 8) * 512
                            if t % 4 == 3:
                                nc.scalar.copy(out=cstage[0:98, off:off + 512], in_=psc[0:98, :])
                            else:
                                nc.vector.tensor_copy(out=cstage[0:98, off:off + 512], in_=psc[0:98, :])
                            if t % 8 == 7:
                                for c in range(4):
                                    nc.scalar.dma_start(
                                        out=outp[c][:, 8 * s8:8 * s8 + 8, :],
                                        in_=cstage[32 * c:32 * c + 2, :].rearrange("p (g d) -> p g d", d=512))
                                del cst[s8]

                for sp2 in range(np_pairs // 2):
                    vtile_ref[0] = vtp.tile([128, 2 * HW_], dt.float16, tag="vt", name="vtile")
                    dmae[dmac[0] % 2].dma_start(out=vtile_ref[0][:], in_=vt[sp2][:])
                    dmac[0] += 1
                    for h2 in range(2):
                        p = 2 * sp2 + h2
                        if p % 8 == 0:
                            z_stage[0] = zstp.tile([2, 8 * TW], dt.float32, tag="zst", name="zstage")
                        emit_pair(sp2, h2)
                    if sp2 % 8 == 7:
                        G = sp2 // 8
                        softmax_batch(G)
                        et_batch(G)
                        for qq in (4 * G, 16 + 4 * G, 4 * G + 1, 16 + 4 * G + 1,
                                   4 * G + 2, 16 + 4 * G + 2, 4 * G + 3, 16 + 4 * G + 3):
                            pass2_slab(qq)

    return nc


def _prep_core(Vc, sc, hc, Wv, Wg, Ws, wh):
    """Build the per-core input map (all layout scrambling lives here)."""
    m = {}
    # V^T slabs: [64, 128, (u, k, 392)] fp16
    VT = Vc.reshape(BL * N, D).T.astype(F16)         # [512, 50176]
    v4 = np.asarray(VT).reshape(NCH, 128, NT, TW)    # [k, dd, t, col]
    slab = np.empty((NP, 128, 2, NCH, TW), dtype=F16)
    for u in range(2):
        tsel = np.arange(NP) + 64 * u
        # [k, dd, p, col] -> [p, dd, k, col]
        slab[:, :, u] = v4[:, :, tsel].transpose(2, 1, 0, 3)
    # pack two pairs per dma slab: [32, 128, (h2, u, k, col)]
    m["vt"] = (slab.reshape(NP // 2, 2, 128, 2 * NCH * TW)
               .transpose(0, 2, 1, 3).reshape(NP // 2, 128, 4 * NCH * TW).copy())

    # natural V rows + s rows: vs[q, r, (g2, c, d)]
    Vhi = Vc.astype(F16)                             # [1024, 49, 512]
    shi = sc.astype(F16)                             # [1024, 512]
    vsarr = np.zeros((64, 100, 2, 4, 512), dtype=F16)
    for g2 in range(2):
        for c in range(4):
            t = 2 * np.arange(64) + g2
            b0 = 8 * t + 2 * c
            vsarr[:, 0:49, g2, c] = np.asarray(Vhi)[b0]
            vsarr[:, 49, g2, c] = np.asarray(shi)[b0]
            vsarr[:, 50:99, g2, c] = np.asarray(Vhi)[b0 + 1]
            vsarr[:, 99, g2, c] = np.asarray(shi)[b0 + 1]
    # pack two q's per dma slab: [32, 100, (j2, g2, c, d)]
    m["vs"] = (vsarr.reshape(32, 2, 100, 4096)
               .transpose(0, 2, 1, 3).reshape(32, 100, 8192).copy())

    # s^T, h^T fp16: [128, (k, b)]
    for name, x in [("st", sc), ("ht", hc)]:
        xt = x.T.astype(F16).reshape(NCH, 128, BL)   # [k, dd, b]
        m[name + "h"] = np.asarray(xt).transpose(1, 0, 2).reshape(128, NCH * BL).copy()

    # weight stationaries: [128, (k, 64)] with cols 49-63 zero
    for name, W in [("wv", Wv), ("wg", Wg), ("ws", Ws)]:
        Wp = np.zeros((D, 64), dtype=F16)
        Wp[:, :N] = W.astype(F16)
        m[name + "h"] = np.asarray(Wp).reshape(NCH, 128, 64).transpose(1, 0, 2).reshape(128, NCH * 64).copy()

    wh2 = np.zeros((128, 2), dtype=F16)
    wh2[0:49, 0] = wh.astype(F16)
    wh2[64:113, 1] = wh.astype(F16)
    m["wh2"] = wh2
    wh1 = np.zeros((64, 1), dtype=F32)
    wh1[0:49, 0] = wh
    m["wh1"] = wh1
    return m


def kernel(V, s_t, h_t, W_v, W_g, W_s, w_h):
    from concourse.bass_utils import run_bass_kernel_spmd

    V = np.asarray(V, dtype=F32)
    s_t = np.asarray(s_t, dtype=F32)
    h_t = np.asarray(h_t, dtype=F32)
    W_v = np.asarray(W_v, dtype=F32)
    W_g = np.asarray(W_g, dtype=F32)
    W_s = np.asarray(W_s, dtype=F32)
    w_h = np.asarray(w_h, dtype=F32)

    nc = build()
    in_maps = []
    for c in range(CORES):
        sl = slice(c * BL, (c + 1) * BL)
        in_maps.append(_prep_core(V[sl], s_t[sl], h_t[sl], W_v, W_g, W_s, w_h))

    res = run_bass_kernel_spmd(nc, in_maps, core_ids=list(range(CORES)), trace=False)

    out = np.empty((B, D), dtype=F32)
    for c in range(CORES):
        o = np.asarray(res.results[c]["out"], dtype=F32)   # [4, 2, 128, 512]
        # b = 8t + 2c' + u  ->  [t, c', u] order
        out[c * BL:(c + 1) * BL] = o.transpose(2, 0, 1, 3).reshape(BL, D)
    return out


if __name__ == "__main__":
    rng = np.random.default_rng(0)
    V = rng.standard_normal((B, N, D), dtype=F32)
    s = rng.standard_normal((B, D), dtype=F32)
    h = rng.standard_normal((B, D), dtype=F32)
    Wv = rng.standard_normal((D, N), dtype=F32)
    Wg = rng.standard_normal((D, N), dtype=F32)
    Ws = rng.standard_normal((D, N), dtype=F32)
    wh = rng.standard_normal(N).astype(F32)
    out = kernel(V=V, s_t=s, h_t=h, W_v=Wv, W_g=Wg, W_s=Ws, w_h=wh)
    print("out", out.shape, out.dtype, np.abs(out).mean())

